# revision 12
# baseline (speedup 1.0000x reference)
"""Trainium2 Bass kernel for a full MHA block (QKV proj + softmax attention +
output proj + residual + LayerNorm), B=2, S=4096, E=512, H=8, D=64.

Sharding: sequence-parallel over 8 cores (4 seq shards x 2 batches). Each core
owns R=1024 query rows of one batch, recomputes K/V for the full context
(avoids all cross-core communication), and writes its own [R, E] output slice.

Layout strategy (per core):
  - x^T via XBAR DMA-transpose (bf16) -> [128, 4, S]
  - K^T, Q^T projections in head-major layout [e_out(=2 heads)/128, t]
  - scores computed transposed: S_T[t, s] = K^T.T @ Q^T, two heads packed into
    PE row groups (K=64 each), PSUM out [t=128, s=512]
  - exp on ScalarE straight from PSUM (scale=1/8 free), out bf16 A_T tiles
  - A@V: lhsT = [V_h | ones] (65 cols) so row 64 of the product accumulates the
    softmax denominator for free; accumulated per 3-chunk group in PSUM then
    drained into an SBUF f32 accumulator by VectorE (keeps PSUM pressure at 8
    banks: 2x3 score staging + 2 utility)
  - normalize: reciprocal of denom row, partition-broadcast via SBUF->SBUF DMA,
    VectorE multiply -> ctx^T bf16 (per-head tiles, base partition 0 always)
  - O-proj: per-head K=64 matmuls accumulating all 8 heads, + residual (f32) +
    LayerNorm (bn_stats/bn_aggr) on VectorE/ScalarE, f32 output.
"""

import sys

sys.path.insert(0, "/opt/trn_rl_repo")

import numpy as np
import ml_dtypes

import concourse.bass as bass
import concourse.bacc as bacc
import concourse.mybir as mybir
import concourse.tile as tile
from concourse.bass import ds, ts

# Problem constants (hardcoded per harness contract)
B = 2
S = 4096
E = 512
H = 8
D = 64
N_CORES = 8
SEQ_SHARDS = N_CORES // B
R = S // SEQ_SHARDS  # 1024 own query rows per core

F32 = mybir.dt.float32
BF16 = mybir.dt.bfloat16
AF = mybir.ActivationFunctionType


def build_mha(nc, seq=S, rows=R, exp_group=2, sblk=512):
    """Emit the Tile program. seq/rows shrinkable for simulation."""
    P = 128
    EC = E // P           # 4 e_in chunks
    HPAIRS = H // 2       # 4 head-pair blocks (=e_out blocks of 128)
    TT = seq // P         # t tiles
    kblk = min(512, seq)
    TB = seq // kblk      # t blocks for K-proj
    qblk = min(512, rows)
    QB = rows // qblk     # r blocks for Q-proj
    sblk = min(sblk, rows)
    SB = rows // sblk     # s blocks per core
    ST = rows // P        # s tiles for O-proj/LN

    # ---- DRAM I/O ----
    x_bf = nc.dram_tensor("x_bf", [seq, E], BF16, kind="ExternalInput").ap()
    xo_bf = nc.dram_tensor("xo_bf", [rows, E], BF16, kind="ExternalInput").ap()
    xo_f32 = nc.dram_tensor("xo_f32", [rows, E], F32, kind="ExternalInput").ap()
    wq = nc.dram_tensor("wq_bf", [E, E], BF16, kind="ExternalInput").ap()
    wk = nc.dram_tensor("wk_bf", [E, E], BF16, kind="ExternalInput").ap()
    wv = nc.dram_tensor("wv_bf", [E, E], BF16, kind="ExternalInput").ap()
    wo = nc.dram_tensor("wo_bf", [E, E], BF16, kind="ExternalInput").ap()
    bq = nc.dram_tensor("bq", [E], F32, kind="ExternalInput").ap()
    bk = nc.dram_tensor("bk", [E], F32, kind="ExternalInput").ap()
    bv = nc.dram_tensor("bv", [E], F32, kind="ExternalInput").ap()
    bo = nc.dram_tensor("bo", [E], F32, kind="ExternalInput").ap()
    ln_g = nc.dram_tensor("ln_g", [E], F32, kind="ExternalInput").ap()
    ln_b = nc.dram_tensor("ln_b", [E], F32, kind="ExternalInput").ap()
    y_out = nc.dram_tensor("y", [rows, E], F32, kind="ExternalOutput").ap()

    with tile.TileContext(nc) as tc:
        with (
            tc.tile_pool(name="singles", bufs=1) as singles,
            tc.tile_pool(name="kqv", bufs=1) as kqv,
            tc.tile_pool(name="vtiles", bufs=TT) as vtiles,
            tc.tile_pool(name="at", bufs=4) as atp,
            tc.tile_pool(name="ctx", bufs=4) as ctxp,
            tc.tile_pool(name="norm", bufs=3) as normp,
            tc.tile_pool(name="yout", bufs=3) as youtp,
            tc.tile_pool(name="dram", bufs=4, space="DRAM") as dramp,
            tc.tile_pool(name="stg", bufs=2, space="PSUM") as stg,
            tc.tile_pool(name="acc", bufs=2, space="PSUM") as accp,
            tc.tile_pool(name="util", bufs=2, space="PSUM") as util,
        ):
            # ---------- constants / weights ----------
            wq_sb = singles.tile([P, EC, E], BF16, name="wq_sb")
            wk_sb = singles.tile([P, EC, E], BF16, name="wk_sb")
            wv_sb = singles.tile([P, EC, E], BF16, name="wv_sb")
            nc.sync.dma_start(wq_sb, wq.rearrange("(c p) e -> p c e", p=P))
            nc.sync.dma_start(wk_sb, wk.rearrange("(c p) e -> p c e", p=P))
            nc.sync.dma_start(wv_sb, wv.rearrange("(c p) e -> p c e", p=P))
            # wo in per-head layout: [64, H, E]
            wo_sb = singles.tile([D, H, E], BF16, name="wo_sb")
            nc.sync.dma_start(wo_sb, wo.rearrange("(h p) e -> p h e", p=D))
            # per-e_out-column biases for k^T/q^T ([128, 4] with col = block)
            bk_sb = singles.tile([P, EC], F32, name="bk_sb")
            bq_sb = singles.tile([P, EC], F32, name="bq_sb")
            nc.sync.dma_start(bk_sb, bk.rearrange("(c p) -> p c", p=P))
            nc.sync.dma_start(bq_sb, bq.rearrange("(c p) -> p c", p=P))
            # free-dim broadcast tiles
            bv_bc = singles.tile([P, E], F32, name="bv_bc")
            bo_bc = singles.tile([P, E], F32, name="bo_bc")
            g_bc = singles.tile([P, E], F32, name="g_bc")
            b_bc = singles.tile([P, E], F32, name="b_bc")
            for dst, src in ((bv_bc, bv), (bo_bc, bo), (g_bc, ln_g), (b_bc, ln_b)):
                nc.gpsimd.dma_start(out=dst, in_=src[None, :].to_broadcast((P, E)))
            eps_t = singles.tile([P, 1], F32, name="eps_t")
            nc.vector.memset(eps_t, 1e-5)

            # ---------- x^T via DMA transpose (per t-block for fast start) ----------
            xoT = singles.tile([P, EC, rows], BF16, name="xoT")
            for c in range(EC):
                nc.sync.dma_start_transpose(xoT[:, c, :], xo_bf[:, ds(c * P, P)])
            xT = singles.tile([P, EC, seq], BF16, name="xT")
            for tb in range(TB):
                for c in range(EC):
                    nc.sync.dma_start_transpose(
                        xT[:, c, ds(tb * kblk, kblk)],
                        x_bf[ds(tb * kblk, kblk), ds(c * P, P)],
                    )

            # ---------- V projection (+bias, +ones col) per t-tile ----------
            # v_aug[t_tile][p, h, 0:64] = v, [..., 64] = 1.0
            v_tiles = []
            def emit_v(t):
                vt = vtiles.tile([P, H, D + 1], BF16, name=f"v_{t}", tag="v")
                nc.vector.memset(vt[:, :, D : D + 1], 1.0)
                ps = util.tile([P, E], F32, name="v_ps", tag="u")
                for c in range(EC):
                    nc.tensor.matmul(
                        ps, lhsT=xT[:, c, ts(t, P)], rhs=wv_sb[:, c, :],
                        start=(c == 0), stop=(c == EC - 1),
                    )
                nc.vector.tensor_add(
                    out=vt[:, :, 0:D],
                    in0=ps.rearrange("p (h d) -> p h d", h=H),
                    in1=bv_bc.rearrange("p (h d) -> p h d", h=H),
                )
                v_tiles.append(vt)

            # ---------- K^T / Q^T projections (per head-pair block) ----------
            kT = [kqv.tile([P, seq], BF16, name=f"kT_{hp}") for hp in range(HPAIRS)]
            qT = [kqv.tile([P, rows], BF16, name=f"qT_{hp}") for hp in range(HPAIRS)]

            def emit_k(hp, tb):
                ps = util.tile([P, 512], F32, name="k_ps", tag="u")
                for c in range(EC):
                    nc.tensor.matmul(
                        ps[:, :kblk], lhsT=wk_sb[:, c, ds(hp * P, P)],
                        rhs=xT[:, c, ds(tb * kblk, kblk)],
                        start=(c == 0), stop=(c == EC - 1),
                    )
                nc.vector.tensor_tensor(
                    kT[hp][:, ds(tb * kblk, kblk)], ps[:, :kblk],
                    bk_sb[:, hp : hp + 1].to_broadcast((P, kblk)),
                    mybir.AluOpType.add,
                )

            def emit_q(hp, rb):
                ps = util.tile([P, 512], F32, name="q_ps", tag="u")
                for c in range(EC):
                    nc.tensor.matmul(
                        ps[:, :qblk], lhsT=wq_sb[:, c, ds(hp * P, P)],
                        rhs=xoT[:, c, ds(rb * qblk, qblk)],
                        start=(c == 0), stop=(c == EC - 1),
                    )
                nc.vector.tensor_tensor(
                    qT[hp][:, ds(rb * qblk, qblk)], ps[:, :qblk],
                    bq_sb[:, hp : hp + 1].to_broadcast((P, qblk)),
                    mybir.AluOpType.add,
                )

            # ---------- attention ----------
            scale = 1.0 / np.sqrt(D)
            # ctx^T accumulators (f32, SBUF), one per head in the pair
            def attention(hp, sb, fillers):
                ctx_ps = [
                    accp.tile([D + 1, sblk], F32, name=f"ctx_{h}", tag="ctx")
                    for h in range(2)
                ]
                n_groups = (TT + exp_group - 1) // exp_group
                for g in range(n_groups):
                    t0 = g * exp_group
                    gsz = min(exp_group, TT - t0)
                    for h in range(2):
                        st_t = stg.tile([P, exp_group, 512], F32, name=f"stg_{h}", tag="stg")
                        # scores (2 heads packed via PE row groups)
                        for j in range(gsz):
                            nc.tensor.matmul(
                                st_t[:, j, :sblk],
                                lhsT=kT[hp][ds(h * D, D), ts(t0 + j, P)],
                                rhs=qT[hp][ds(h * D, D), ds(sb * sblk, sblk)],
                                start=True, stop=True,
                                tile_position=(h * D, 0),
                            )
                        # exp (with 1/sqrt(D) folded in), PSUM -> SBUF bf16
                        at_t = atp.tile([P, exp_group, 512], BF16, name=f"at_{h}", tag="at")
                        nc.scalar.activation(
                            out=at_t[:, :gsz, :sblk], in_=st_t[:, :gsz, :sblk],
                            func=AF.Exp, scale=scale,
                        )
                        # A@V accumulated in a pinned PSUM bank across all groups
                        for j in range(gsz):
                            nc.tensor.matmul(
                                ctx_ps[h][:, :sblk],
                                lhsT=v_tiles[t0 + j][:, hp * 2 + h, :],
                                rhs=at_t[:, j, :sblk],
                                start=(g == 0 and j == 0),
                                stop=(g == n_groups - 1 and j == gsz - 1),
                            )
                    if fillers:
                        fillers.pop(0)()
                ctx_sb = ctx_ps
                # normalize: ctxT_h = ctx_sb[0:64] * (1/denom) broadcast.
                # Reciprocal runs on a [128, x] reshape (all DVE lanes) via a
                # DRAM bounce; a second bounce broadcasts it across partitions.
                for h in range(2):
                    fw = sblk // P
                    dnr = normp.tile([D + 1, sblk], F32, name="dnr")
                    nc.vector.tensor_copy(
                        dnr[D : D + 1, :], ctx_sb[h][D : D + 1, :]
                    )
                    dr = dramp.tile([sblk], F32, name="dr", tag="dr")
                    nc.sync.dma_start(out=dr[None, :], in_=dnr[D : D + 1, :])
                    dn4 = normp.tile([P, fw], F32, name="dn4")
                    nc.sync.dma_start(
                        out=dn4, in_=dr.rearrange("(p f) -> p f", p=P)
                    )
                    nc.vector.reciprocal(out=dn4, in_=dn4)
                    dr2 = dramp.tile([sblk], F32, name="dr2", tag="dr2")
                    nc.sync.dma_start(
                        out=dr2.rearrange("(p f) -> p f", p=P), in_=dn4
                    )
                    rb_t = normp.tile([D, sblk], F32, name="rb")
                    nc.gpsimd.dma_start(
                        out=rb_t, in_=dr2[None, :].to_broadcast((D, sblk))
                    )
                    nc.vector.tensor_mul(
                        out=ctxT[hp * 2 + h][:, ds(sb * sblk, sblk)],
                        in0=ctx_sb[h][0:D, :], in1=rb_t,
                    )

            ctxT = [kqv.tile([D, rows], BF16, name=f"ctxT_{h}") for h in range(H)]

            # ---------- O-projection + residual + LayerNorm ----------
            def emit_out(st):
                ps = util.tile([P, E], F32, name="o_ps", tag="u")
                for h in range(H):
                    nc.tensor.matmul(
                        ps, lhsT=ctxT[h][:, ts(st, P)], rhs=wo_sb[:, h, :],
                        start=(h == 0), stop=(h == H - 1),
                    )
                xo_t = youtp.tile([P, E], F32, name="xo_t")
                nc.sync.dma_start(xo_t, xo_f32[ts(st, P), :])
                y_t = youtp.tile([P, E], F32, name="y_t")
                nc.vector.tensor_add(out=y_t, in0=ps, in1=xo_t)
                nc.vector.tensor_add(out=y_t, in0=y_t, in1=bo_bc)
                # LayerNorm
                stats = normp.tile([P, 6], F32, name="stats")
                nc.vector.bn_stats(out=stats, in_=y_t)
                mv = normp.tile([P, 2], F32, name="mv")
                nc.vector.bn_aggr(out=mv, in_=stats)
                std = normp.tile([P, 1], F32, name="std")
                nc.scalar.activation(
                    out=std, in_=mv[:, 1:2], func=AF.Sqrt, bias=eps_t
                )
                nc.vector.reciprocal(out=std, in_=std)
                nc.vector.tensor_tensor(
                    y_t, y_t, mv[:, 0:1].to_broadcast((P, E)),
                    mybir.AluOpType.subtract,
                )
                nc.vector.tensor_tensor(
                    y_t, y_t, std.to_broadcast((P, E)), mybir.AluOpType.mult,
                )
                nc.vector.tensor_mul(out=y_t, in0=y_t, in1=g_bc)
                nc.vector.tensor_add(out=y_t, in0=y_t, in1=b_bc)
                nc.sync.dma_start(y_out[ts(st, P), :], y_t)


            # ---------- emission order ----------
            # K(hp0)/Q(hp0) first so ScalarE starts ASAP; V per t-block after
            # each K block so A@V stays just-ahead.
            emitted_out = set()
            tpb = kblk // P  # t-tiles per k-block
            for tb in range(TB):
                emit_k(0, tb)
                if tb == 0:
                    for rb in range(QB):
                        emit_q(0, rb)
                for t in range(tb * tpb, (tb + 1) * tpb):
                    emit_v(t)

            for hp in range(HPAIRS):
                fillers = []
                if hp + 1 < HPAIRS:
                    nhp = hp + 1
                    for tb in range(TB):
                        fillers.append(lambda nhp=nhp, tb=tb: emit_k(nhp, tb))
                    for rb in range(QB):
                        fillers.append(lambda nhp=nhp, rb=rb: emit_q(nhp, rb))
                elif SB > 1:
                    # last head-pair: stream first s-block's output tiles
                    def of(st):
                        def run():
                            emit_out(st)
                            emitted_out.add(st)
                        return run
                per_sb = (len(fillers) + 1) // SB if fillers else 0
                for sb in range(SB):
                    if hp == HPAIRS - 1 and sb == SB - 1 and SB > 1:
                        chunk = [of(st) for st in range(ST // SB)]
                    else:
                        chunk = fillers[:per_sb]
                        del fillers[:per_sb]
                    attention(hp, sb, chunk)
                    for f in chunk:
                        f()
                for f in fillers:
                    f()

            for st in range(ST):
                if st not in emitted_out:
                    emit_out(st)

    return nc


_CACHED = {}


def _get_nc(seq=S, rows=R, exp_group=2, sblk=512):
    key = (seq, rows, exp_group, sblk)
    if key not in _CACHED:
        nc = bacc.Bacc("TRN2", target_bir_lowering=False, debug=False,
                       num_devices=N_CORES)
        build_mha(nc, seq=seq, rows=rows, exp_group=exp_group, sblk=sblk)
        nc.compile()
        _CACHED[key] = nc
    return _CACHED[key]


def make_in_maps(inputs):
    """Shard full inputs into per-core input dicts."""
    bf = ml_dtypes.bfloat16
    x = np.asarray(inputs["x"], np.float32)
    shared = {
        "wq_bf": np.asarray(inputs["wq"], bf),
        "wk_bf": np.asarray(inputs["wk"], bf),
        "wv_bf": np.asarray(inputs["wv"], bf),
        "wo_bf": np.asarray(inputs["wo"], bf),
        "bq": np.asarray(inputs["bq"], np.float32),
        "bk": np.asarray(inputs["bk"], np.float32),
        "bv": np.asarray(inputs["bv"], np.float32),
        "bo": np.asarray(inputs["bo"], np.float32),
        "ln_g": np.asarray(inputs["ln_g"], np.float32),
        "ln_b": np.asarray(inputs["ln_b"], np.float32),
    }
    x_bf_all = [np.ascontiguousarray(x[b].astype(bf)) for b in range(B)]
    in_maps = []
    for c in range(N_CORES):
        b, shard = divmod(c, SEQ_SHARDS)
        r0 = shard * R
        m = dict(shared)
        m["x_bf"] = x_bf_all[b]
        m["xo_bf"] = np.ascontiguousarray(x_bf_all[b][r0 : r0 + R])
        m["xo_f32"] = np.ascontiguousarray(x[b, r0 : r0 + R])
        in_maps.append(m)
    return in_maps


def kernel(**inputs):
    from concourse import bass_utils

    nc = _get_nc()
    in_maps = make_in_maps(inputs)
    res = bass_utils.run_bass_kernel_spmd(nc, in_maps, core_ids=list(range(N_CORES)))
    out = np.empty((B, S, E), np.float32)
    for c in range(N_CORES):
        b, shard = divmod(c, SEQ_SHARDS)
        out[b, shard * R : (shard + 1) * R] = res.results[c]["y"]
    return out
